# revision 6
# baseline (speedup 1.0000x reference)
"""Trainium2 Bass kernel for GNN message calculation:

    messages = H[heads] @ Wh.T + E @ We.T + b

Sharding: edges (m) data-parallel across 8 cores; H (node table) and W
replicated per core. Host-side prep is layout-only (transpose/pad/permute):
the gather and all GEMMs run on device.

Device layout (per core, shard of MS=100000 edges padded to MP=100352):
  - h   [100000, 128] f32 : full node table (gather source, row-major)
  - et  [128, MP]     f32 : E-shard transposed (d on partitions)
  - hw  [128, MP/128] i32 : heads in wrap layout, hw[p, i] = heads[i*128 + p]
  - wt  [128, 256]    f32 : [Wh.T | We.T]  (d on partitions, k on free)
  - bv  [128, 1]      f32 : bias (k on partitions)
  - out [128, MP]     f32 : messages^T

Per 512-edge block: indirect-DMA gather 512 H rows -> 4 PE transposes into
one PSUM bank -> DVE copy to SBUF -> 2 accumulating fp32 matmuls with the
128x128 weight halves stationary (N=512) -> ACT bias+copy -> DMA out.
"""

import math
from contextlib import ExitStack

import numpy as np

import concourse.bacc as bacc
import concourse.bass as bass
import concourse.tile as tile
from concourse import mybir
from concourse.bass_utils import run_bass_kernel_spmd
from concourse.masks import make_identity

P = 128
D = 128
N_NODES = 100000
M_EDGES = 800000
N_CORES = 8
MS = M_EDGES // N_CORES          # edges per shard
BLK = 512                        # edges per block (one PSUM bank at f32)
MP = ((MS + BLK - 1) // BLK) * BLK   # padded shard size (100352)
NB = MP // BLK                   # blocks per shard (196)
NT = MP // P                     # 128-edge tiles per shard (784)
KPG = BLK // P                   # gathered rows per partition per block (4)

F32 = mybir.dt.float32
I32 = mybir.dt.int32


def build_nc(mp=MP):
    """Build the Bass module. mp must be a multiple of BLK."""
    nb = mp // BLK
    nt = mp // P

    nc = bacc.Bacc("TRN2", debug=False, num_devices=N_CORES)

    h_d = nc.dram_tensor("h", (N_NODES, D), F32, kind="ExternalInput").ap()
    et_d = nc.dram_tensor("et", (D, mp), F32, kind="ExternalInput").ap()
    hw_d = nc.dram_tensor("hw", (P, nt), I32, kind="ExternalInput").ap()
    wt_d = nc.dram_tensor("wt", (D, 2 * D), F32, kind="ExternalInput").ap()
    bv_d = nc.dram_tensor("bv", (D, 1), F32, kind="ExternalInput").ap()
    out_d = nc.dram_tensor("out", (D, mp), F32, kind="ExternalOutput").ap()

    with ExitStack() as ctx:
        tc = ctx.enter_context(tile.TileContext(nc, num_cores=N_CORES))
        cpool = ctx.enter_context(tc.tile_pool(name="const", bufs=1))
        iop = ctx.enter_context(tc.tile_pool(name="io", bufs=3))
        psp = ctx.enter_context(tc.tile_pool(name="ps", bufs=2, space="PSUM"))

        w_s = cpool.tile([D, 2 * D], F32)
        nc.sync.dma_start(out=w_s[:], in_=wt_d[:, :])
        b_s = cpool.tile([D, 1], F32)
        nc.sync.dma_start(out=b_s[:], in_=bv_d[:, :])
        ident = cpool.tile([P, P], F32)
        make_identity(nc, ident[:])
        heads_s = cpool.tile([P, nt], I32)
        nc.sync.dma_start(out=heads_s[:], in_=hw_d[:, :])

        for g in range(nb):
            # gather H rows for this block's 512 edges:
            # gh[p, j*128:(j+1)*128] = H[hw[p, g*KPG + j], :]
            # NB: HW indirect DMA consumes exactly ONE index per partition and
            # reads the partition's full dest size contiguously, so each
            # 128-row sub-tile needs its own instruction with [128,1] offsets.
            gh = iop.tile([P, BLK], F32, tag="gh")
            for j in range(KPG):
                t = g * KPG + j
                nc.gpsimd.indirect_dma_start(
                    out=gh[:, j * P:(j + 1) * P],
                    out_offset=None,
                    in_=h_d[:, :],
                    in_offset=bass.IndirectOffsetOnAxis(
                        ap=heads_s[:, t:t + 1], axis=0
                    ),
                )

            # transpose each 128x128 sub-tile: trp[:, j*128+p] = edge j*128+p's features
            trp = psp.tile([P, BLK], F32, tag="trp")
            for j in range(KPG):
                nc.tensor.transpose(
                    out=trp[:, j * P:(j + 1) * P],
                    in_=gh[:, j * P:(j + 1) * P],
                    identity=ident[:],
                )
            th = iop.tile([P, BLK], F32, tag="th")
            nc.vector.tensor_copy(out=th[:], in_=trp[:])

            et = iop.tile([P, BLK], F32, tag="et")
            nc.sync.dma_start(out=et[:], in_=et_d[:, g * BLK:(g + 1) * BLK])

            # po[k, m] = sum_d Wh[k,d] * Hh[m,d] + sum_d We[k,d] * E[m,d]
            po = psp.tile([P, BLK], F32, tag="po")
            nc.tensor.matmul(
                out=po[:], lhsT=w_s[:, 0:D], rhs=th[:], start=True, stop=False
            )
            nc.tensor.matmul(
                out=po[:], lhsT=w_s[:, D:2 * D], rhs=et[:], start=False, stop=True
            )

            # bias add (per-partition) + PSUM -> SBUF
            ob = iop.tile([P, BLK], F32, tag="ob")
            nc.scalar.activation(
                out=ob[:], in_=po[:],
                func=mybir.ActivationFunctionType.Identity,
                bias=b_s[:, 0:1],
            )
            nc.sync.dma_start(out=out_d[:, g * BLK:(g + 1) * BLK], in_=ob[:])

    nc.compile()
    return nc


_NC_CACHE = {}


def _get_nc(mp=MP):
    if mp not in _NC_CACHE:
        _NC_CACHE[mp] = build_nc(mp)
    return _NC_CACHE[mp]


def _prep_core_inputs(H, E, heads, W, b, s, ms=MS, mp=MP):
    """Host-side layout prep for shard s (pure reshape/transpose/pad)."""
    nt = mp // P
    Es = E[s * ms:(s + 1) * ms]
    hs = heads[s * ms:(s + 1) * ms]

    et = np.zeros((D, mp), dtype=np.float32)
    et[:, :ms] = Es.T

    hp = np.zeros(mp, dtype=np.int32)
    hp[:ms] = hs
    hw = np.ascontiguousarray(hp.reshape(nt, P).T)

    wt = np.ascontiguousarray(
        np.concatenate([W[:, :D].T, W[:, D:].T], axis=1), dtype=np.float32
    )
    return {
        "h": H,
        "et": et,
        "hw": hw,
        "wt": wt,
        "bv": np.ascontiguousarray(b.reshape(D, 1), dtype=np.float32),
    }


def kernel(H, E, heads, queries, W, b):
    H = np.ascontiguousarray(np.asarray(H), dtype=np.float32)
    E = np.ascontiguousarray(np.asarray(E), dtype=np.float32)
    heads = np.asarray(heads).astype(np.int32)
    W = np.ascontiguousarray(np.asarray(W), dtype=np.float32)
    b = np.asarray(b, dtype=np.float32)

    nc = _get_nc()
    in_maps = [
        _prep_core_inputs(H, E, heads, W, b, s) for s in range(N_CORES)
    ]
    res = run_bass_kernel_spmd(nc, in_maps, list(range(N_CORES)))

    out = np.empty((M_EDGES, D), dtype=np.float32)
    for s in range(N_CORES):
        out[s * MS:(s + 1) * MS] = res.results[s]["out"][:, :MS].T
    return out


# revision 8
# speedup vs baseline: 1.0198x; 1.0198x over previous
"""Trainium2 Bass kernel for GNN message calculation:

    messages = H[heads] @ Wh.T + E @ We.T + b

H: (100000, 128) f32, E: (800000, 128) f32, heads: (800000,) int, W: (128, 256),
b: (128,). Output (800000, 128) f32.

Strategy
--------
Edges are sharded data-parallel across 8 cores (100000/core); H and W are
replicated. All gathers and GEMMs run on device; host work is layout only
(transpose/pad/permute/dtype-split).

Per core:
  * Edges are grouped into 4 bins by node-index range (bin = head >> 15) so
    each bin's gathers address < 32768 table rows -> int16 local indices for
    the fast vectorized `dma_gather` SWDGE path (the generic indirect-DMA
    path costs ~1.1us of GPSIMD time per 128 rows; dma_gather does 1024 rows
    per instruction). Bins get fixed block capacities so the program is
    identical on every core (SPMD); unused slots are padded with index 0 /
    zero E rows and dropped on the host side.
  * H is stored as fp16 hi/lo pairs (row = [hi(128) | lo(128)], 512B) so one
    512B-row gather fetches both halves; E likewise ships as fp16 hi/lo.
  * The gathered [m, d] tiles are transposed to [d, m] with the DMA XBAR
    transpose (2-byte dtype), not the PE.
  * messages^T is computed in PSUM as 6 accumulating fp16 matmuls per
    512-edge block (3-term Karatsuba-style split per operand pair:
    W@X ~= Whi@Xhi + Wlo@Xhi + Whi@Xlo, dropped term ~2^-22 relative), plus
    a per-partition bias add on the way out. fp16 runs the PE at 1 cyc/row
    vs fp32's ~6-8.
"""

import numpy as np

import concourse.bacc as bacc
import concourse.tile as tile
from concourse import mybir
from concourse.bass_utils import run_bass_kernel_spmd

P = 128
D = 128
N_NODES = 100000
M_EDGES = 800000
N_CORES = 8
MS = M_EDGES // N_CORES      # edges per shard

SEG = 32768                  # table rows per bin (int16-addressable)
N_BINS = 4                   # ceil(100000 / 32768)
GBLK = 1024                  # edges per dma_gather instruction
BIN_GATHERS = [33, 33, 33, 3]   # fixed per-bin gather capacity (SPMD static)
NG = sum(BIN_GATHERS)        # 102 gathers
MP = NG * GBLK               # 104448 device edge slots per shard
BLK = 512                    # edges per PSUM block
NB = MP // BLK               # 204 blocks
IDXC = GBLK // 16            # idx columns per gather (64)

SEG_OF_G = sum(([b] * n for b, n in enumerate(BIN_GATHERS)), [])
GATHER_START = np.cumsum([0] + BIN_GATHERS).tolist()  # [0, 33, 66, 99, 102]

F32 = mybir.dt.float32
F16 = mybir.dt.float16
I16 = mybir.dt.int16


def build_nc():
    nc = bacc.Bacc("TRN2", debug=False, num_devices=N_CORES)

    h16_d = nc.dram_tensor("h16", (N_NODES, 2 * D), F16, kind="ExternalInput").ap()
    et_d = nc.dram_tensor("et16", (P, NB * 2 * BLK), F16, kind="ExternalInput").ap()
    ix_d = nc.dram_tensor("ix16", (P, NG * IDXC), I16, kind="ExternalInput").ap()
    w_d = nc.dram_tensor("w16", (P, 4 * D), F16, kind="ExternalInput").ap()
    bv_d = nc.dram_tensor("bv", (P, 1), F32, kind="ExternalInput").ap()
    out_d = nc.dram_tensor("out", (P, MP), F32, kind="ExternalOutput").ap()

    seg_rows = [min(SEG, N_NODES - b * SEG) for b in range(N_BINS)]

    from contextlib import ExitStack
    with ExitStack() as ctx:
        tc = ctx.enter_context(tile.TileContext(nc, num_cores=N_CORES))
        cpool = ctx.enter_context(tc.tile_pool(name="const", bufs=1))
        iop = ctx.enter_context(tc.tile_pool(name="io", bufs=3))
        psp = ctx.enter_context(tc.tile_pool(name="ps", bufs=4, space="PSUM"))

        w_s = cpool.tile([P, 4 * D], F16)
        nc.sync.dma_start(out=w_s[:], in_=w_d[:, :])
        b_s = cpool.tile([P, 1], F32)
        nc.sync.dma_start(out=b_s[:], in_=bv_d[:, :])
        ix_s = cpool.tile([P, NG * IDXC], I16)
        nc.sync.dma_start(out=ix_s[:], in_=ix_d[:, :])

        for g in range(NG):
            seg = SEG_OF_G[g]
            h_seg = h16_d[seg * SEG: seg * SEG + seg_rows[seg], :]

            # gather 1024 rows: gh[p, j, :] = [hi|lo] of edge (g*1024 + j*128 + p)
            gh = iop.tile([P, GBLK // P, 2 * D], F16, tag="gh")
            nc.gpsimd.dma_gather(
                out_ap=gh[:],
                in_ap=h_seg,
                idxs_ap=ix_s[:, g * IDXC:(g + 1) * IDXC],
                num_idxs=GBLK,
                num_idxs_reg=GBLK,
                elem_size=2 * D,
            )

            # XBAR transpose the whole gather at once:
            # xp[d, 2j+h, m] = gh[m, j, h*128 + d]  (h=0: hi, h=1: lo)
            xp = iop.tile([P, 2 * GBLK // P, P], F16, tag="xp")
            nc.scalar.dma_start(
                out=xp[:], in_=gh[:].rearrange("p j e -> p (j e)"), transpose=True
            )
            xpr = xp[:].rearrange("d (j two) m -> d two j m", two=2)

            for blk in range(GBLK // BLK):
                bb = g * (GBLK // BLK) + blk
                j0 = blk * (BLK // P)
                rhs_hi = xpr[:, 0, j0:j0 + BLK // P, :]
                rhs_lo = xpr[:, 1, j0:j0 + BLK // P, :]

                et = iop.tile([P, 2 * BLK], F16, tag="et")
                nc.sync.dma_start(
                    out=et[:], in_=et_d[:, bb * 2 * BLK:(bb + 1) * 2 * BLK]
                )

                po = psp.tile([P, BLK], F32, tag="po")
                # H-part: Whhi@Hhi + Whlo@Hhi + Whhi@Hlo
                nc.tensor.matmul(out=po[:], lhsT=w_s[:, 0:D], rhs=rhs_hi,
                                 start=True, stop=False)
                nc.tensor.matmul(out=po[:], lhsT=w_s[:, D:2 * D], rhs=rhs_hi,
                                 start=False, stop=False)
                nc.tensor.matmul(out=po[:], lhsT=w_s[:, 0:D], rhs=rhs_lo,
                                 start=False, stop=False)
                # E-part: Wehi@Ehi + Welo@Ehi + Wehi@Elo
                nc.tensor.matmul(out=po[:], lhsT=w_s[:, 2 * D:3 * D],
                                 rhs=et[:, 0:BLK], start=False, stop=False)
                nc.tensor.matmul(out=po[:], lhsT=w_s[:, 3 * D:4 * D],
                                 rhs=et[:, 0:BLK], start=False, stop=False)
                nc.tensor.matmul(out=po[:], lhsT=w_s[:, 2 * D:3 * D],
                                 rhs=et[:, BLK:2 * BLK], start=False, stop=True)

                # bias add (per-partition) + PSUM -> SBUF on DVE
                ob = iop.tile([P, BLK], F32, tag="ob")
                nc.vector.tensor_scalar_add(out=ob[:], in0=po[:],
                                            scalar1=b_s[:, 0:1])
                nc.sync.dma_start(
                    out=out_d[:, bb * BLK:(bb + 1) * BLK], in_=ob[:]
                )

    nc.compile()
    return nc


_NC_CACHE = {}


def _get_nc():
    if "nc" not in _NC_CACHE:
        _NC_CACHE["nc"] = build_nc()
    return _NC_CACHE["nc"]


def _split_f16(x):
    hi = x.astype(np.float16)
    lo = (x - hi.astype(np.float32)).astype(np.float16)
    return hi, lo


def _prep_shared(H, W, b):
    """Replicated tensors (same on every core)."""
    h16 = np.empty((N_NODES, 2 * D), dtype=np.float16)
    h16[:, :D], h16[:, D:] = _split_f16(H)

    w16 = np.empty((P, 4 * D), dtype=np.float16)
    WhT = np.ascontiguousarray(W[:, :D].T)
    WeT = np.ascontiguousarray(W[:, D:].T)
    w16[:, 0:D], w16[:, D:2 * D] = _split_f16(WhT)
    w16[:, 2 * D:3 * D], w16[:, 3 * D:4 * D] = _split_f16(WeT)

    bv = np.ascontiguousarray(b.reshape(P, 1).astype(np.float32))
    return h16, w16, bv


def _prep_shard(E, heads, s):
    """Per-shard layout prep: bin edges by head>>15, pad bins to capacity,
    build device-side E (fp16 hi/lo, transposed), int16 gather indices, and
    the original-position -> device-slot map."""
    Es = E[s * MS:(s + 1) * MS]
    hs = heads[s * MS:(s + 1) * MS].astype(np.int64)

    bins = (hs >> 15).astype(np.int64)
    order = np.argsort(bins, kind="stable")
    counts = np.bincount(bins, minlength=N_BINS)
    caps = [n * GBLK for n in BIN_GATHERS]
    if not all(counts[i] <= caps[i] for i in range(N_BINS)):
        raise RuntimeError(f"bin overflow: counts={counts} caps={caps}")

    starts = [GATHER_START[i] * GBLK for i in range(N_BINS)]
    dev_pos = np.empty(MS, dtype=np.int64)
    hloc = np.zeros(MP, dtype=np.int16)
    ehi = np.zeros((MP, D), dtype=np.float16)
    elo = np.zeros((MP, D), dtype=np.float16)
    cum = 0
    for bin_ in range(N_BINS):
        cnt = int(counts[bin_])
        sel = order[cum:cum + cnt]
        cum += cnt
        sl = slice(starts[bin_], starts[bin_] + cnt)
        dev_pos[sel] = np.arange(starts[bin_], starts[bin_] + cnt)
        hloc[sl] = (hs[sel] - (bin_ << 15)).astype(np.int16)
        Eb = Es[sel]
        hi = Eb.astype(np.float16)
        ehi[sl] = hi
        elo[sl] = (Eb - hi.astype(np.float32)).astype(np.float16)

    # et16 [128, NB*1024]: block bb cols [bb*1024, bb*1024+512) = Ehi^T,
    # [bb*1024+512, ...) = Elo^T
    et = np.stack(
        [ehi.reshape(NB, BLK, D), elo.reshape(NB, BLK, D)], axis=1
    )  # [NB, 2, BLK, D]
    et16 = np.ascontiguousarray(et.transpose(3, 0, 1, 2).reshape(P, NB * 2 * BLK))

    # ix16 [128, NG*64]: per gather g the 1024 int16 indices wrapped in 16
    # partitions (element k -> [k%16, k//16]), replicated 8x down partitions.
    x = hloc.reshape(NG, IDXC, 16).transpose(0, 2, 1)  # [NG, 16, 64]
    row16 = x.transpose(1, 0, 2).reshape(16, NG * IDXC)
    ix16 = np.ascontiguousarray(np.tile(row16, (8, 1)))

    return et16, ix16, dev_pos


def kernel(H, E, heads, queries, W, b):
    H = np.ascontiguousarray(np.asarray(H), dtype=np.float32)
    E = np.ascontiguousarray(np.asarray(E), dtype=np.float32)
    heads = np.asarray(heads)
    W = np.ascontiguousarray(np.asarray(W), dtype=np.float32)
    b = np.asarray(b, dtype=np.float32)

    nc = _get_nc()
    h16, w16, bv = _prep_shared(H, W, b)
    in_maps = []
    dev_positions = []
    for s in range(N_CORES):
        et16, ix16, dev_pos = _prep_shard(E, heads, s)
        dev_positions.append(dev_pos)
        in_maps.append(
            {"h16": h16, "et16": et16, "ix16": ix16, "w16": w16, "bv": bv}
        )

    res = run_bass_kernel_spmd(nc, in_maps, list(range(N_CORES)))

    out = np.empty((M_EDGES, D), dtype=np.float32)
    for s in range(N_CORES):
        out[s * MS:(s + 1) * MS] = res.results[s]["out"][:, dev_positions[s]].T
    return out
